# revision 27
# baseline (speedup 1.0000x reference)
"""Trainium2 Bass kernel for MultiHeadedAttention + residual + LayerNorm.

Problem: B=2, S=2048, D=1024, H=16 heads (DK=64), fp32 in/out.
  q,k,v = (x @ W + b) per projection; per-head scaled-dot-product attention
  with full S x S score matrix; out = LayerNorm(attn_out + query) * gamma + beta.

Sharding (8 NeuronCores, tensor-parallel over heads):
  Core c owns heads {2c, 2c+1} == output channels [128c, 128c+128).

Structure per core (vs. the earlier baseline):
  - Q/K projections computed transposed (qT/kT = W.T @ x.T); the two heads'
    score matmuls contract over 64 partitions each and run CONCURRENTLY via
    PE row-tiling (tile_position auto-derived from base_partition 0 / 64).
  - V projection computed token-major directly (lhsT = x k-tile), removing
    the 32 PE transposes + 64 DVE copies the transposed V path needed.
    v-bias is folded into the residual host-side (softmax rows sum to 1).
  - exp(s/8) is split between ACT (table exp) and DVE (Schraudolph int16
    bit-trick producing bf16 directly, ~+-3% rel err on a tunable subset of
    key-tiles) so the 16.8M-element softmax doesn't serialize on ACT alone.
  - AV keeps the ones-column trick ([v|1], M=65 per head) so the softmax
    denominator accumulates for free in PSUM row 64.
  - Output transposes write [128,4,65] PSUM tiles per (tq, head); one
    reciprocal per 4 chunks; y kept bf16 for 2x DVE modes; LN stats are raw
    (sum y, sum y^2) per 128-ch shard, AllReduced (16KB) across the 8 cores,
    batch 0's LN tail overlapping batch 1's attention.
  - Emission order software-pipelines: k/q chunk projections and V groups of
    the *other* batch are interleaved between 4-key-tile attention groups as
    PE filler, so the PE never idles while ACT grinds exp and ACT starts
    ~6us into the kernel (warmup exp preloads the table at t=0).
Host assembles the 8 channel slices into the full (2, 2048, 1024) output.
"""

import numpy as np

B, S, D, H, DK = 2, 2048, 1024, 16, 64
T = B * S              # 4096 flattened tokens
NCORES = 8
NCH = D // NCORES      # 128 channels (2 heads) per core
KT = D // 128          # 8 contraction tiles for projections
NTILE = T // 128       # 32 token tiles of 128
ST = S // 128          # 16 key tiles per batch
TQ = S // 512          # 4 query chunks of 512 per batch

# key-tiles whose exp runs on DVE (Schraudolph) instead of ACT, per (b, tq)
DVE_ST = (5, 11, 14)
# bf16 Schraudolph: i16 = s * EXPA + EXPB; bits reinterpreted as bf16 give
# ~exp(s/8) with ~+-3% max rel error. EXPA = 128*log2(e)/8.
EXPA = 23.083120654223414
EXPB = 16250.5
# fp8 DoubleRow AV: attention probabilities and V in fp8e5 (e4m3 overflows
# to inf at 240 for heavy-tail scores), two key-tiles contracted per matmul
# (2 fp8 weights/PE cell). exp computes exp(s/8 - 2): softmax shift-
# invariance cancels the -2 in the denominator division.
AV_FP8 = True
EXPA8 = 0.7213475204444817
EXPB8 = 48.28
# emission-order software pipelining (False = baseline-style serial order)
PIPELINE = True

_COMPILED = None


def _build_program(with_collective: bool = True, repeat: int = 1):
    import concourse.bass as bass
    import concourse.mybir as mybir
    import concourse.tile as tile
    from concourse import bacc
    from concourse.masks import make_identity

    F32 = mybir.dt.float32
    BF16 = mybir.dt.bfloat16
    I16 = mybir.dt.int16
    I32 = mybir.dt.int32
    U8 = mybir.dt.uint8
    FP8 = mybir.dt.float8e5
    FP8V = mybir.dt.float8e4  # V fits e4m3 (+-240); 3-bit mantissa halves its quant error
    AF = mybir.ActivationFunctionType
    DR = mybir.MatmulPerfMode.DoubleRow

    nc = bacc.Bacc(
        "TRN2",
        target_bir_lowering=False,
        debug=False,
        enable_asserts=False,
        num_devices=NCORES,
    )

    xqT_d = nc.dram_tensor("xqT", (D, T), BF16, kind="ExternalInput")
    xkT_d = nc.dram_tensor("xkT", (D, T), BF16, kind="ExternalInput")
    xvT_d = nc.dram_tensor("xvT", (D, T), BF16, kind="ExternalInput")
    wq_d = nc.dram_tensor("wq", (KT, 128, NCH), BF16, kind="ExternalInput")
    wk_d = nc.dram_tensor("wk", (KT, 128, NCH), BF16, kind="ExternalInput")
    wv_d = nc.dram_tensor("wv", (KT, 128, NCH), BF16, kind="ExternalInput")
    bq_d = nc.dram_tensor("bq", (NCH, 1), F32, kind="ExternalInput")
    bk_d = nc.dram_tensor("bk", (NCH, 1), F32, kind="ExternalInput")
    res_d = nc.dram_tensor("resid", (NTILE, 128, NCH), BF16, kind="ExternalInput")
    gam_d = nc.dram_tensor("gamma", (1, NCH), BF16, kind="ExternalInput")
    bet_d = nc.dram_tensor("beta", (1, NCH), BF16, kind="ExternalInput")
    out_d = nc.dram_tensor("out", (NTILE, 128, NCH), F32, kind="ExternalOutput")

    with tile.TileContext(nc) as tc:
        with (
            tc.tile_pool(name="const", bufs=1) as const,
            tc.tile_pool(name="big", bufs=1) as big,
            tc.tile_pool(name="xin", bufs=48) as xin,
            tc.tile_pool(name="ptp", bufs=3) as ptp,
            tc.tile_pool(name="otp", bufs=2) as otp,
            tc.tile_pool(name="rpool", bufs=2) as rpool,
            tc.tile_pool(name="ypool", bufs=2) as ypool,
            tc.tile_pool(name="small", bufs=6) as small,
            tc.tile_pool(name="sppool", bufs=2, space="PSUM") as sppool,
            tc.tile_pool(name="ovpool", bufs=1, space="PSUM") as ovpool,
            tc.tile_pool(name="pjpool", bufs=1, space="PSUM") as pjpool,
            tc.tile_pool(name="tppool", bufs=1, space="PSUM") as tppool,
            tc.tile_pool(name="dram", bufs=1, space="DRAM") as dram,
        ):
            identb = const.tile([128, 128], BF16)
            make_identity(nc, identb[:])

            # weights + biases loaded once
            wts, bts = {}, {}
            for nm, w_dram, b_dram in (
                ("q", wq_d, bq_d), ("k", wk_d, bk_d), ("v", wv_d, None),
            ):
                w = const.tile([128, KT, NCH], BF16, tag="w" + nm, name="w" + nm)
                nc.sync.dma_start(w[:], w_dram.ap().rearrange("kt p m -> p kt m"))
                wts[nm] = w
                if b_dram is not None:
                    bt = const.tile([NCH, 1], F32, tag="b" + nm, name="b" + nm)
                    nc.sync.dma_start(bt[:], b_dram[:])
                    bts[nm] = bt

            gam = const.tile([128, NCH], BF16)
            nc.sync.dma_start(
                gam[:],
                bass.AP(tensor=gam_d.ap().tensor, offset=0, ap=[[0, 128], [1, NCH]]),
            )
            bet = const.tile([128, NCH], BF16)
            nc.sync.dma_start(
                bet[:],
                bass.AP(tensor=bet_d.ap().tensor, offset=0, ap=[[0, 128], [1, NCH]]),
            )

            nb2 = const.tile([128, 1], F32, tag="nb2", name="nb2")
            nc.vector.memset(nb2[:], -2.0)
            wu_in = const.tile([1, 2], F32, tag="wui", name="wui")
            wu_out = const.tile([1, 2], BF16, tag="wuo", name="wuo")
            nc.vector.memset(wu_in[:], 0.0)

            def load_x(xT_dram, b):
                """One [128,512] tile per (k-tile, 512-token chunk). DMAs for
                chunk c are emitted by load_chunk(c) so the caller controls
                queue order (rings are FIFOs: bytes enqueued ahead of a tile
                gate when its consumers can start)."""
                xs = [[None] * 4 for _ in range(KT)]

                def load_chunk(c):
                    for kt in range(KT):
                        xc = xin.tile([128, 512], BF16, tag="xa", name="xa")
                        nc.sync.dma_start(
                            xc[:],
                            xT_dram[
                                kt * 128 : (kt + 1) * 128,
                                b * S + c * 512 : b * S + (c + 1) * 512,
                            ],
                        )
                        xs[kt][c] = xc

                def sl(kt, c0, c1):
                    # columns [c0, c1) of batch-b token range (chunk-local)
                    c = c0 // 512
                    return xs[kt][c][:, c0 - c * 512 : c1 - c * 512]

                return sl, load_chunk

            def proj_chunk(nm, xt, n, dest):
                """dest[:, n*512:(n+1)*512] = W.T @ x + b  (transposed layout)"""
                w, bt = wts[nm], bts[nm]
                ps = pjpool.tile([128, 512], F32, tag="pj", name="pjps")
                for kt in range(KT):
                    nc.tensor.matmul(
                        ps[:], w[:, kt, :], xt(kt, n * 512, (n + 1) * 512),
                        start=(kt == 0), stop=(kt == KT - 1),
                    )
                nc.vector.tensor_scalar_add(
                    dest[:, n * 512 : (n + 1) * 512], ps[:], bt[:]
                )

            def vgroup(xt, g, v130):
                """Token-major V projection for key-tiles 4g..4g+3 straight
                into the [v_h0 | 1 | v_h1 | 1] AV operand layout. With AV_FP8
                the destination is the DoubleRow-interleaved [pair, j] tile."""
                wv = wts["v"]
                ps = pjpool.tile([128, 512], F32, tag="pj", name="vps")
                for j in range(4):
                    for kt in range(KT):
                        nc.tensor.matmul(
                            ps[:, j * 128 : (j + 1) * 128],
                            xt(kt, g * 512 + j * 128, g * 512 + (j + 1) * 128),
                            wv[:, kt, :],
                            start=(kt == 0), stop=(kt == KT - 1),
                        )
                psv = ps[:].rearrange("p (j c) -> p j c", c=128)
                if AV_FP8:
                    lo = v130[:, 2 * g : 2 * g + 2, :, 0:64]
                    hi = v130[:, 2 * g : 2 * g + 2, :, 65:129]
                else:
                    lo = v130[:, 4 * g : 4 * g + 4, 0:64]
                    hi = v130[:, 4 * g : 4 * g + 4, 65:129]
                nc.vector.tensor_copy(lo, psv[:, :, 0:64])
                nc.vector.tensor_copy(hi, psv[:, :, 64:128])

            def scores_mm(tq, st, qT, kT_):
                sp = sppool.tile([128, 1024], F32, tag="sp", name="sp")
                t0 = tq * 512
                k0 = st * 128
                for h in range(2):
                    hs = slice(h * 64, (h + 1) * 64)
                    nc.tensor.matmul(
                        sp[:, h * 512 : (h + 1) * 512],
                        kT_[hs, k0 : k0 + 128],
                        qT[hs, t0 : t0 + 512],
                        start=True, stop=True,
                    )
                return sp

            def attn_st(tq, st, qT, kT_, v130, op):
                sp = scores_mm(tq, st, qT, kT_)
                pt = ptp.tile([128, 1024], BF16, tag="pt", name="pt")
                if st in DVE_ST:
                    nc.vector.tensor_scalar(
                        pt[:].bitcast(I16), sp[:], EXPA, EXPB,
                        op0=mybir.AluOpType.mult, op1=mybir.AluOpType.add,
                    )
                else:
                    nc.scalar.activation(pt[:], sp[:], AF.Exp, scale=0.125)
                for h in range(2):
                    nc.tensor.matmul(
                        op[:, h, :],
                        v130[:, st, h * 65 : (h + 1) * 65],
                        pt[:, h * 512 : (h + 1) * 512],
                        start=(st == 0), stop=(st == ST - 1),
                    )

            def attn_pair(tq, spi, qT, kT_, v2, op):
                """fp8 DoubleRow: two key-tiles (2*spi, 2*spi+1) per AV matmul,
                probabilities exp(s/8 - 2) in fp8e4 (shift cancels in the
                denominator division)."""
                pt2 = ptp.tile([128, 2, 1024], FP8, tag="pt2", name="pt2")
                for j in range(2):
                    st = 2 * spi + j
                    sp = scores_mm(tq, st, qT, kT_)
                    if st in DVE_ST:
                        nc.vector.tensor_scalar(
                            pt2[:, j, :].bitcast(U8), sp[:], EXPA8, EXPB8,
                            op0=mybir.AluOpType.mult, op1=mybir.AluOpType.add,
                        )
                    else:
                        nc.scalar.activation(
                            pt2[:, j, :], sp[:], AF.Exp, scale=0.125, bias=nb2[:]
                        )
                for h in range(2):
                    nc.tensor.matmul(
                        op[:, h, :],
                        v2[:, spi, :, h * 65 : (h + 1) * 65],
                        pt2[:, :, h * 512 : (h + 1) * 512],
                        start=(spi == 0), stop=(spi == ST // 2 - 1),
                        perf_mode=DR,
                    )

            def rt_prefetch(b, tq):
                rt = rpool.tile([128, 4, NCH], BF16, tag="rt", name="rt")
                nc.sync.dma_start(
                    rt[:],
                    res_d.ap()[
                        b * ST + tq * 4 : b * ST + tq * 4 + 4
                    ].rearrange("n p m -> p n m"),
                )
                return rt

            def attn_tail(b, tq, op, y_all, stats, rt):
                oT = otp.tile([65, 2, 512], BF16, tag="oT", name="oT")
                nc.vector.tensor_copy(oT[:], op[:])
                for h in range(2):
                    # inner dim padded 65->66 so each q4 chunk starts 4B-aligned
                    tp = tppool.tile([128, 4, 66], BF16, tag="tp", name="tp")
                    for q4 in range(4):
                        nc.tensor.transpose(
                            tp[:, q4, 0:65],
                            oT[:, h, q4 * 128 : (q4 + 1) * 128],
                            identb[0:65, 0:65],
                        )
                    rc = small.tile([128, 4], F32, tag="rc", name="rc")
                    nc.vector.reciprocal(rc[:], tp[:, :, 64])
                    for q4 in range(4):
                        nc.vector.tensor_scalar_mul(
                            y_all[:, tq * 4 + q4, h * 64 : (h + 1) * 64],
                            tp[:, q4, 0:64],
                            rc[:, q4 : q4 + 1],
                        )
                ysl = y_all[:, tq * 4 : tq * 4 + 4, :]
                nc.vector.tensor_add(ysl, ysl, rt[:])
                sq = rpool.tile([128, 4, NCH], BF16, tag="sq", name="sq")
                nc.vector.tensor_mul(sq[:], ysl, ysl)
                nc.vector.tensor_reduce(
                    stats[:, tq * 4 : tq * 4 + 4, 0], ysl,
                    axis=mybir.AxisListType.X, op=mybir.AluOpType.add,
                )
                nc.vector.tensor_reduce(
                    stats[:, tq * 4 : tq * 4 + 4, 1], sq[:],
                    axis=mybir.AxisListType.X, op=mybir.AluOpType.add,
                )

            def ln_tail(b, y_all, stats, tqs, sfx):
                # AllReduce (sum y, sum y^2) for token-chunks `tqs` across the
                # 8 cores. b1's LN runs as two halves so the first half's
                # normalize+store overlaps the second half's attention.
                nh = len(tqs) * 4
                i0 = tqs[0] * 4
                tg = f"{b}{sfx}"
                cin = dram.tile([128, nh, 2], F32, tag=f"cin{tg}", name=f"cin{tg}")
                cout = dram.tile([128, nh, 2], F32, tag=f"cout{tg}", name=f"cout{tg}")
                nc.sync.dma_start(cin[:], stats[:, i0 : i0 + nh, :])
                if with_collective:
                    nc.gpsimd.collective_compute(
                        "AllReduce",
                        mybir.AluOpType.add,
                        replica_groups=[list(range(NCORES))],
                        ins=[cin.opt()],
                        outs=[cout.opt()],
                    )
                else:  # timeline-sim variant: collective unsupported there
                    nc.sync.dma_start(cout[:], cin[:])
                ssum = big.tile([128, nh, 2], F32, tag=f"ss{tg}", name=f"ss{tg}")
                nc.sync.dma_start(ssum[:], cout[:])

                # 1/D scaling on DVE, NOT ScalarE: ACT's queue is strict FIFO,
                # so a ScalarE op waiting on the collective would stall the
                # next batch's exp stream behind it.
                mu = big.tile([128, nh], F32, tag=f"mu{tg}", name=f"mu{tg}")
                nc.vector.tensor_scalar_mul(mu[:], ssum[:, :, 0], 1.0 / D)
                e2 = small.tile([128, nh], F32, tag="e2", name="e2")
                nc.vector.tensor_scalar_mul(e2[:], ssum[:, :, 1], 1.0 / D)
                musq = small.tile([128, nh], F32, tag="musq", name="musq")
                nc.vector.tensor_mul(musq[:], mu[:], mu[:])
                av = big.tile([128, nh], F32, tag=f"av{tg}", name=f"av{tg}")
                nc.vector.tensor_sub(av[:], e2[:], musq[:])
                nc.vector.tensor_scalar_add(av[:], av[:], 1e-6)
                # rstd = rsqrt(av) on DVE only (ACT Sqrt would thrash the exp
                # table set, ~2.7us per reload): exponent-halving seed via
                # integer shifts, then 5 Newton iterations to fp32 accuracy.
                rst = big.tile([128, nh], F32, tag=f"rst{tg}", name=f"rst{tg}")
                ei = small.tile([128, nh], I32, tag="ei", name="ei")
                nc.vector.tensor_scalar(
                    ei[:], av[:].bitcast(I32), 23, None,
                    op0=mybir.AluOpType.logical_shift_right,
                )
                nc.vector.tensor_scalar(
                    ei[:], ei[:], -1, 381,
                    op0=mybir.AluOpType.mult, op1=mybir.AluOpType.add,
                )
                nc.vector.tensor_scalar(
                    ei[:], ei[:], 1, None,
                    op0=mybir.AluOpType.logical_shift_right,
                )
                nc.vector.tensor_scalar(
                    rst[:].bitcast(I32), ei[:], 23, None,
                    op0=mybir.AluOpType.logical_shift_left,
                )
                r2 = small.tile([128, nh], F32, tag="r2", name="r2")
                for _newton in range(5):
                    nc.vector.tensor_mul(r2[:], rst[:], rst[:])
                    nc.vector.tensor_mul(r2[:], r2[:], av[:])
                    nc.vector.tensor_scalar(
                        r2[:], r2[:], -0.5, 1.5,
                        op0=mybir.AluOpType.mult, op1=mybir.AluOpType.add,
                    )
                    nc.vector.tensor_mul(rst[:], rst[:], r2[:])

                gamb = gam[:].rearrange("p (o c) -> p o c", o=1).broadcast_to(
                    [128, 4, NCH]
                )
                betb = bet[:].rearrange("p (o c) -> p o c", o=1).broadcast_to(
                    [128, 4, NCH]
                )
                for tq in tqs:
                    ysl = y_all[:, tq * 4 : tq * 4 + 4, :]
                    for q4 in range(4):
                        idx = tq * 4 + q4
                        r = idx - i0
                        yv = y_all[:, idx, :]
                        nc.vector.tensor_scalar(
                            yv, yv, mu[:, r : r + 1], rst[:, r : r + 1],
                            op0=mybir.AluOpType.subtract, op1=mybir.AluOpType.mult,
                        )
                    nc.vector.tensor_mul(ysl, ysl, gamb)
                    yo = ypool.tile([128, 4, NCH], F32, tag="yo", name="yo")
                    nc.vector.tensor_add(yo[:], ysl, betb)
                    nc.sync.dma_start(
                        out_d.ap()[
                            b * ST + tq * 4 : b * ST + tq * 4 + 4
                        ].rearrange("n p m -> p n m"),
                        yo[:],
                    )

            def one_pass(first):
                if first:
                    # preload the exp table set during the initial DMAs
                    nc.scalar.activation(wu_out[:], wu_in[:], AF.Exp, scale=0.125)

                xts, qkT, v130s, ystats = {}, {}, {}, {}
                for b in range(B):
                    qkT[b] = (
                        big.tile([128, S], BF16, tag=f"qT{b}", name=f"qT{b}"),
                        big.tile([128, S], BF16, tag=f"kT{b}", name=f"kT{b}"),
                    )
                    if AV_FP8:
                        # [pair, j, 144]: inner padded 130->144 so the DR
                        # interleave (Ko) step is a multiple of 16 bytes
                        v130s[b] = big.tile(
                            [128, ST // 2, 2, 144], FP8V,
                            tag=f"v130_{b}", name=f"v130_{b}",
                        )
                    else:
                        v130s[b] = big.tile(
                            [128, ST, 130], BF16, tag=f"v130_{b}", name=f"v130_{b}"
                        )
                    ystats[b] = (
                        big.tile([128, ST, NCH], BF16, tag=f"y{b}", name=f"y{b}"),
                        big.tile([128, ST, 2], F32, tag=f"st{b}", name=f"st{b}"),
                    )

                def load_b(b):
                    loaders = {}
                    xts[b] = {}
                    for nm, dram_t in (("k", xkT_d), ("q", xqT_d), ("v", xvT_d)):
                        xts[b][nm], loaders[nm] = load_x(dram_t, b)
                    # DMA queue order = consumption order: chunk-0 of k/q/v
                    # first (unblocks scores at ~3MB), then k/v chunks (needed
                    # within tq0), q chunks last (needed from tq1 on).
                    for nm, c in (
                        ("k", 0), ("q", 0), ("v", 0),
                        ("k", 1), ("v", 1), ("k", 2), ("k", 3),
                        ("q", 1), ("v", 2), ("v", 3), ("q", 2), ("q", 3),
                    ):
                        loaders[nm](c)
                    if AV_FP8:
                        nc.vector.memset(v130s[b][:, :, :, 64:65], 1.0)
                        nc.vector.memset(v130s[b][:, :, :, 129:130], 1.0)
                    else:
                        nc.vector.memset(v130s[b][:, :, 64:65], 1.0)
                        nc.vector.memset(v130s[b][:, :, 129:130], 1.0)

                # filler[b][tq][g]: projection work for emission between
                # attention key-tile groups; program order sets PE priority.
                def qc(b, n):
                    return lambda: proj_chunk("q", xts[b]["q"], n, qkT[b][0])

                def kc(b, n):
                    return lambda: proj_chunk("k", xts[b]["k"], n, qkT[b][1])

                def vg(b, g):
                    return lambda: vgroup(xts[b]["v"], g, v130s[b])

                # vg(b, g) MUST be emitted before group g's attn_st calls:
                # Tile deps bind a reader to the writes emitted before it.
                fill = {
                    0: {
                        0: [[vg(0, 0)], [kc(0, 1), vg(0, 1)],
                            [kc(0, 2), vg(0, 2)], [kc(0, 3), vg(0, 3)], []],
                        1: [[qc(0, 1)], [], [], [], []],
                        2: [[qc(0, 2)], [qc(0, 3)], [], [], []],
                        3: [[lambda: load_b(1)], [kc(1, 0)], [qc(1, 0)],
                            [kc(1, 1)], [vg(1, 0)]],
                    },
                    1: {
                        0: [[kc(1, 2)], [kc(1, 3), vg(1, 1)], [vg(1, 2)],
                            [vg(1, 3)], []],
                        1: [[qc(1, 1)], [], [], [], []],
                        2: [[qc(1, 2)], [], [], [], []],
                        3: [[qc(1, 3)], [], [], [], []],
                    },
                }

                load_b(0)
                proj_chunk("k", xts[0]["k"], 0, qkT[0][1])
                proj_chunk("q", xts[0]["q"], 0, qkT[0][0])
                for b in range(B):
                    qT, kT_ = qkT[b]
                    y_all, stats = ystats[b]
                    for tq in range(TQ):
                        op = ovpool.tile([65, 2, 512], F32, tag="op", name="op")
                        rt = rt_prefetch(b, tq)
                        for g in range(4):
                            for f in fill[b][tq][g]:
                                f()
                            if AV_FP8:
                                for spi in range(2 * g, 2 * g + 2):
                                    attn_pair(tq, spi, qT, kT_, v130s[b], op)
                            else:
                                for st in range(4 * g, 4 * g + 4):
                                    attn_st(tq, st, qT, kT_, v130s[b], op)
                        for f in fill[b][tq][4]:
                            f()
                        attn_tail(b, tq, op, y_all, stats, rt)
                        if b == 1 and tq == 1:
                            ln_tail(1, y_all, stats, (0, 1), "h0")
                        if b == 1 and tq == 2:
                            ln_tail(1, y_all, stats, (2,), "h1")
                    if b == 0:
                        ln_tail(0, y_all, stats, (0, 1, 2, 3), "")
                    else:
                        ln_tail(1, y_all, stats, (3,), "h2")

            for rep in range(repeat):
                one_pass(rep == 0)

    nc.compile()
    return nc


def _get_compiled():
    global _COMPILED
    if _COMPILED is None:
        _COMPILED = _build_program()
    return _COMPILED


def _make_in_maps(query, key_, value, Wq, bq, Wk, bk, Wv, bv, ln_gamma, ln_beta):
    import ml_dtypes

    f = np.float32
    bf = ml_dtypes.bfloat16
    q2 = np.ascontiguousarray(query.reshape(T, D), dtype=f)
    xqT = np.ascontiguousarray(q2.T).astype(bf)
    xkT = np.ascontiguousarray(key_.reshape(T, D).T, dtype=f).astype(bf)
    xvT = np.ascontiguousarray(value.reshape(T, D).T, dtype=f).astype(bf)
    resid_full = q2 + np.asarray(bv, dtype=f)[None, :]  # v-bias folded in
    in_maps = []
    for c in range(NCORES):
        sl = slice(NCH * c, NCH * (c + 1))
        in_maps.append({
            "xqT": xqT,
            "xkT": xkT,
            "xvT": xvT,
            "wq": np.ascontiguousarray(Wq[:, sl], dtype=f).reshape(KT, 128, NCH).astype(bf),
            "wk": np.ascontiguousarray(Wk[:, sl], dtype=f).reshape(KT, 128, NCH).astype(bf),
            "wv": np.ascontiguousarray(Wv[:, sl], dtype=f).reshape(KT, 128, NCH).astype(bf),
            "bq": np.ascontiguousarray(bq[sl], dtype=f).reshape(NCH, 1),
            "bk": np.ascontiguousarray(bk[sl], dtype=f).reshape(NCH, 1),
            "resid": np.ascontiguousarray(resid_full[:, sl]).reshape(NTILE, 128, NCH).astype(bf),
            "gamma": np.ascontiguousarray(ln_gamma[sl], dtype=f).reshape(1, NCH).astype(bf),
            "beta": np.ascontiguousarray(ln_beta[sl], dtype=f).reshape(1, NCH).astype(bf),
        })
    return in_maps


def kernel(query, key_, value, Wq, bq, Wk, bk, Wv, bv, ln_gamma, ln_beta):
    from concourse import bass_utils

    nc = _get_compiled()
    in_maps = _make_in_maps(
        query, key_, value, Wq, bq, Wk, bk, Wv, bv, ln_gamma, ln_beta
    )
    res = bass_utils.run_bass_kernel_spmd(nc, in_maps, core_ids=list(range(NCORES)))
    slices = [res.results[c]["out"].reshape(T, NCH) for c in range(NCORES)]
    out = np.concatenate(slices, axis=1)
    return out.reshape(B, S, D)
